# revision 4
# baseline (speedup 1.0000x reference)
"""Trainium2 Bass kernel for NT-Xent contrastive loss (BATCH=4096, DIM=512, TEMP=0.5).

Strategy (data-parallel over rows of the 2B x 2B similarity matrix):
  - Host: E = concat(emb_i, emb_j) [8192, 512] f32, cast bf16. Each core gets
    ET = E.T (replicated) + its own / partner 1024-column blocks of ET, plus
    row-major copies (full + own + partner) for cheap DVE row-norms.
  - Device (per core, SPMD, no collectives):
      * sumsq of every row via DVE scalar_tensor_tensor square+accumulate
        over the row-major tiles -> r = 1/||e|| (ACT sqrt in two batches,
        DVE reciprocal)
      * normalize the rhs copy column-wise: z_j = e_j * r_j (r broadcast via
        DRAM-bounce + step-0 DMA)
      * S' = e_block^T @ Z on PE (bf16, fp32 accum), [128, 1024] PSUM groups
      * ACT: exp(S' * r_row/TEMP) with fused row-sum accumulation
      * positives via DVE row-dots of own x partner row-major blocks
      * per-core partial: sum_rows(log(den - e^{1/TEMP}) - pos/TEMP) -> [1,1]
  - Host: loss = sum(partials) / (2B).
"""

import math

import ml_dtypes
import numpy as np

BATCH = 4096
DIM = 512
TEMP = 0.5
B2 = 2 * BATCH              # 8192 rows/cols of the similarity matrix
NCORES = 8
RPC = B2 // NCORES          # 1024 rows per core
KT = DIM // 128             # 4 contraction chunks
CG = 8                      # column groups
CGW = B2 // CG              # 1024 columns per group
T8 = RPC // 128             # 8 row-tiles per group / per core
NBF = CGW // 512            # 512-wide matmuls per group
EXP_DIAG = math.exp(1.0 / TEMP)
SQ_SPLIT = 2                # column groups covered by the first sqrt batch

_CACHE = {}


def _build():
    import concourse.bass as bass
    import concourse.bacc as bacc
    import concourse.mybir as mybir
    import concourse.tile as tile

    f32 = mybir.dt.float32
    bf16 = mybir.dt.bfloat16
    AF = mybir.ActivationFunctionType
    ALU = mybir.AluOpType
    X = mybir.AxisListType.X

    nc = bacc.Bacc("TRN2", target_bir_lowering=False, debug=False,
                   num_devices=NCORES)

    et_d = nc.dram_tensor("et", [DIM, B2], bf16, kind="ExternalInput").ap()
    etb_d = nc.dram_tensor("etb", [DIM, RPC], bf16, kind="ExternalInput").ap()
    etp_d = nc.dram_tensor("etp", [DIM, RPC], bf16, kind="ExternalInput").ap()
    erm_d = nc.dram_tensor("erm", [B2, DIM], bf16, kind="ExternalInput").ap()
    ermb_d = nc.dram_tensor("ermb", [RPC, DIM], bf16, kind="ExternalInput").ap()
    ermp_d = nc.dram_tensor("ermp", [RPC, DIM], bf16, kind="ExternalInput").ap()
    out_d = nc.dram_tensor("out", [1, 1], f32, kind="ExternalOutput").ap()
    rflat = [nc.dram_tensor(f"rflat{c}", [CGW], bf16) for c in range(CG)]

    def rm_load(pool, dram_slice, name, tag=None, bufs=None):
        """Load [1024, 512] row-major DRAM rows into a [128, 8*512] tile
        (row-tile-major: tile[p, t*512+d] = rows[t*128+p, d])."""
        kw = {}
        if tag is not None:
            kw = dict(tag=tag, bufs=bufs)
        sb = pool.tile([128, T8 * DIM], bf16, name=name, **kw)
        nc.sync.dma_start(
            sb[:].rearrange("p (t d) -> p t d", d=DIM),
            dram_slice.rearrange("(t p) d -> p t d", p=128))
        return sb

    with tile.TileContext(nc) as tc:
        with (
            tc.tile_pool(name="persist", bufs=1) as P,
            tc.tile_pool(name="scratch", bufs=2) as S,
            tc.tile_pool(name="psum", bufs=4, space="PSUM") as PS,
        ):
            ss64 = P.tile([128, 64], f32, name="ss64")
            ssb = P.tile([128, T8], f32, name="ssb")
            ssp = P.tile([128, T8], f32, name="ssp")
            rawpos = P.tile([128, T8], f32, name="rawpos")
            rsums = P.tile([128, T8 * CG], f32, name="rsums")
            sc8 = P.tile([128, T8], f32, name="sc8")
            pos8 = P.tile([128, T8], f32, name="pos8")
            ones = P.tile([128, 1], f32, name="ones")
            nc.vector.memset(ones[:], 1.0)

            # ---- DMAs + sumsq, interleaved for early availability ----
            ermb = rm_load(S, ermb_d[:, :], "ermb")
            ermp = rm_load(S, ermp_d[:, :], "ermp")
            etb = [P.tile([128, RPC], bf16, name=f"etb_{k}") for k in range(KT)]
            etp = [P.tile([128, RPC], bf16, name=f"etp_{k}") for k in range(KT)]
            for k in range(KT):
                nc.sync.dma_start(etb[k][:], etb_d[k * 128:(k + 1) * 128, :])
                nc.sync.dma_start(etp[k][:], etp_d[k * 128:(k + 1) * 128, :])

            def sumsq(src, tt, dst, dcol):
                sco = S.tile([128, DIM], bf16, tag="stt", name="sco")
                nc.vector.scalar_tensor_tensor(
                    sco[:], src[:, tt * DIM:(tt + 1) * DIM], 1.0,
                    src[:, tt * DIM:(tt + 1) * DIM], ALU.mult, ALU.mult,
                    accum_out=dst[:, dcol:dcol + 1])

            for t in range(T8):
                sumsq(ermb, t, ssb, t)
                sumsq(ermp, t, ssp, t)
            # raw positives: row-dots of own x partner
            for t in range(T8):
                sco = S.tile([128, DIM], bf16, tag="stt", name="sco")
                nc.vector.scalar_tensor_tensor(
                    sco[:], ermb[:, t * DIM:(t + 1) * DIM], 1.0,
                    ermp[:, t * DIM:(t + 1) * DIM], ALU.mult, ALU.mult,
                    accum_out=rawpos[:, t:t + 1])

            # full-row sumsq per column group; et chunks interleaved
            erm = [None] * CG
            et = [[None] * CG for _ in range(KT)]

            def load_erm(c):
                erm[c] = rm_load(S, erm_d[c * CGW:(c + 1) * CGW, :],
                                 f"erm_{c}", tag="erm", bufs=2)
                for tt in range(T8):
                    sumsq(erm[c], tt, ss64, c * 8 + tt)

            def load_et(c):
                for k in range(KT):
                    et[k][c] = P.tile([128, CGW], bf16, name=f"et_{k}_{c}")
                    nc.sync.dma_start(
                        et[k][c][:],
                        et_d[k * 128:(k + 1) * 128, c * CGW:(c + 1) * CGW])

            load_erm(0)
            load_erm(1)
            load_et(0)
            for c in range(2, CG):
                load_erm(c)
                load_et(c - 1)
            load_et(CG - 1)

            # ---- own/partner norms -> exp scale + positives ----
            nb8 = S.tile([128, T8], f32, tag="nrm", name="nb8")
            nc.scalar.activation(nb8[:], ssb[:], AF.Sqrt)
            rb8 = P.tile([128, T8], f32, name="rb8")
            nc.vector.reciprocal(rb8[:], nb8[:])
            np8 = S.tile([128, T8], f32, tag="nrm", name="np8")
            nc.scalar.activation(np8[:], ssp[:], AF.Sqrt)
            rp8 = P.tile([128, T8], f32, name="rp8")
            nc.vector.reciprocal(rp8[:], np8[:])
            nc.vector.tensor_scalar_mul(sc8[:], rb8[:], 1.0 / TEMP)
            pt0 = P.tile([128, T8], f32, name="pt0")
            nc.vector.tensor_mul(pt0[:], rawpos[:], rb8[:])
            pt1 = P.tile([128, T8], f32, name="pt1")
            nc.vector.tensor_mul(pt1[:], pt0[:], rp8[:])
            nc.vector.tensor_scalar_mul(pos8[:], pt1[:], 1.0 / TEMP)

            # ---- column norms in two sqrt batches; broadcast + normalize ----
            rbc = [P.tile([128, CGW], bf16, name=f"rbc_{c}") for c in range(CG)]

            def rchain(c0, c1):
                w = (c1 - c0) * T8
                nsq = S.tile([128, w], f32, tag=f"nsq{c0}", name="nsq")
                nc.scalar.activation(nsq[:], ss64[:, c0 * 8:c1 * 8], AF.Sqrt)
                rcp = S.tile([128, w], f32, tag=f"rcp{c0}", name="rcp")
                nc.vector.reciprocal(rcp[:], nsq[:])
                rcb = S.tile([128, w], bf16, tag=f"rcb{c0}", name="rcb")
                nc.vector.tensor_copy(rcb[:], rcp[:])
                for c in range(c0, c1):
                    off = (c - c0) * T8
                    nc.sync.dma_start(
                        bass.AP(rflat[c], 0, [[1, 128], [128, T8]]),
                        rcb[:, off:off + T8])
                    nc.sync.dma_start(rbc[c][:],
                                      bass.AP(rflat[c], 0, [[0, 128], [1, CGW]]))
                    for k in range(KT):
                        nc.vector.tensor_mul(et[k][c][:], et[k][c][:],
                                             rbc[c][:])

            rchain(0, SQ_SPLIT)
            rchain(SQ_SPLIT, CG)

            # ---- main loop: similarity row-block + fused exp/row-sum ----
            for c in range(CG):
                for t in range(T8):
                    ps = PS.tile([128, CGW], f32, tag="mm", name="psmm")
                    for k in range(KT):
                        for n in range(NBF):
                            nc.tensor.matmul(
                                ps[:, n * 512:(n + 1) * 512],
                                etb[k][:, t * 128:(t + 1) * 128],
                                et[k][c][:, n * 512:(n + 1) * 512],
                                start=(k == 0), stop=(k == KT - 1))
                    sce = S.tile([128, CGW], bf16, tag="expout", name="sce")
                    col = t * CG + c
                    nc.scalar.activation(sce[:], ps[:], AF.Exp,
                                         scale=sc8[:, t:t + 1],
                                         accum_out=rsums[:, col:col + 1])

            # ---- finalize: den = rowsum - e^{1/T}; sum(log(den) - pos) ----
            den8 = P.tile([128, T8], f32, name="den8")
            nc.vector.tensor_reduce(
                den8[:], rsums[:].rearrange("p (t c) -> p t c", c=CG),
                X, ALU.add)
            den8b = P.tile([128, T8], f32, name="den8b")
            nc.vector.tensor_scalar_add(den8b[:], den8[:], -EXP_DIAG)
            logd = S.tile([128, T8], f32, tag="logd", name="logd")
            tlog = P.tile([128, 1], f32, name="tlog")
            nc.scalar.activation(logd[:], den8b[:], AF.Ln, accum_out=tlog[:])
            tpos = P.tile([128, 1], f32, name="tpos")
            nc.vector.tensor_reduce(tpos[:], pos8[:], X, ALU.add)
            lv = P.tile([128, 1], f32, name="lv")
            nc.vector.tensor_sub(lv[:], tlog[:], tpos[:])
            psf = PS.tile([1, 1], f32, tag="mm", name="psf")
            nc.tensor.matmul(psf[:], lv[:], ones[:], start=True, stop=True)
            ob = P.tile([1, 1], f32, name="ob")
            nc.vector.tensor_copy(ob[:], psf[:])
            nc.sync.dma_start(out_d[:], ob[:])

    nc.compile()
    return nc


def _get_nc():
    if "nc" not in _CACHE:
        _CACHE["nc"] = _build()
    return _CACHE["nc"]


def _in_maps(emb_i, emb_j):
    bf = ml_dtypes.bfloat16
    E = np.concatenate([np.asarray(emb_i, dtype=np.float32),
                        np.asarray(emb_j, dtype=np.float32)], axis=0)
    Ebf = E.astype(bf)                              # [8192, 512] row-major
    ET = np.ascontiguousarray(Ebf.T)                # [512, 8192]
    maps = []
    for k in range(NCORES):
        s = k * RPC
        p = (s + BATCH) % B2
        maps.append({
            "et": ET,
            "etb": np.ascontiguousarray(ET[:, s:s + RPC]),
            "etp": np.ascontiguousarray(ET[:, p:p + RPC]),
            "erm": Ebf,
            "ermb": np.ascontiguousarray(Ebf[s:s + RPC, :]),
            "ermp": np.ascontiguousarray(Ebf[p:p + RPC, :]),
        })
    return maps


def _run(emb_i, emb_j, trace=False):
    from concourse.bass_utils import run_bass_kernel_spmd
    nc = _get_nc()
    res = run_bass_kernel_spmd(nc, _in_maps(emb_i, emb_j),
                               list(range(NCORES)), trace=trace)
    total = sum(float(res.results[i]["out"][0, 0]) for i in range(NCORES))
    loss = np.float32(total / B2)
    return loss, res


def kernel(emb_i, emb_j):
    return _run(emb_i, emb_j, trace=False)[0]


# revision 7
# speedup vs baseline: 1.0175x; 1.0175x over previous
"""Trainium2 Bass kernel for NT-Xent contrastive loss (BATCH=4096, DIM=512, TEMP=0.5).

Strategy (data-parallel over rows of the 2B x 2B similarity matrix):
  - Host: E = concat(emb_i, emb_j) [8192, 512] f32, cast bf16. Each core gets
    ET = E.T (replicated) + its own / partner 1024-column blocks of ET, plus
    row-major copies (full + own + partner) for cheap DVE row-norms.
  - Device (per core, SPMD, no collectives):
      * sumsq of every row via DVE scalar_tensor_tensor square+accumulate
        over row-major tiles -> r = 1/||e|| (ACT sqrt in three batches paced
        with the DMA stream, DVE reciprocal)
      * normalize the rhs copy column-wise in place: z_j = e_j * r_j
        (r broadcast via DRAM-bounce + step-0 DMA)
      * S' = e_block^T @ Z on PE (bf16, fp32 accum), [128, 2048] PSUM groups
        (two column groups per exp)
      * ACT: exp(S' * r_row/TEMP) with fused row-sum accumulation
      * positives via DVE row-dots of own x partner row-major blocks
      * per-core partial: sum_rows(log(den - e^{1/TEMP}) - pos/TEMP) -> [1,1]
  - Host: loss = sum(partials) / (2B).

Emission order is deliberate: it sets per-engine queue order, pacing the
normalization pipeline (DMA -> DVE sumsq -> ACT sqrt -> DVE recip/cast ->
DMA broadcast -> DVE normalize) just ahead of the PE/ACT main-loop stream.
"""

import math

import ml_dtypes
import numpy as np

BATCH = 4096
DIM = 512
TEMP = 0.5
B2 = 2 * BATCH              # 8192 rows/cols of the similarity matrix
NCORES = 8
RPC = B2 // NCORES          # 1024 rows per core
KT = DIM // 128             # 4 contraction chunks
CG = 8                      # column groups
CGW = B2 // CG              # 1024 columns per group
T8 = RPC // 128             # 8 row-tiles per group / per core
NBF = CGW // 512            # 512-wide matmuls per group
NCP = CG // 2               # column-group pairs (exp granularity)
EXP_DIAG = math.exp(1.0 / TEMP)

_CACHE = {}


def _build():
    import concourse.bass as bass
    import concourse.bacc as bacc
    import concourse.mybir as mybir
    import concourse.tile as tile

    f32 = mybir.dt.float32
    bf16 = mybir.dt.bfloat16
    AF = mybir.ActivationFunctionType
    ALU = mybir.AluOpType
    X = mybir.AxisListType.X

    nc = bacc.Bacc("TRN2", target_bir_lowering=False, debug=False,
                   num_devices=NCORES)

    et_d = nc.dram_tensor("et", [DIM, B2], bf16, kind="ExternalInput").ap()
    etb_d = nc.dram_tensor("etb", [DIM, RPC], bf16, kind="ExternalInput").ap()
    erm_d = nc.dram_tensor("erm", [B2, DIM], bf16, kind="ExternalInput").ap()
    ermb_d = nc.dram_tensor("ermb", [RPC, DIM], bf16, kind="ExternalInput").ap()
    ermp_d = nc.dram_tensor("ermp", [RPC, DIM], bf16, kind="ExternalInput").ap()
    out_d = nc.dram_tensor("out", [1, 1], f32, kind="ExternalOutput").ap()
    rflat = [nc.dram_tensor(f"rflat{c}", [CGW], bf16) for c in range(CG)]

    with tile.TileContext(nc) as tc:
        with (
            tc.tile_pool(name="persist", bufs=1) as P,
            tc.tile_pool(name="scratch", bufs=2) as S,
            tc.tile_pool(name="psum", bufs=2, space="PSUM") as PS,
        ):
            ss64 = P.tile([128, 64], f32, name="ss64")
            ssb = P.tile([128, T8], f32, name="ssb")
            ssp = P.tile([128, T8], f32, name="ssp")
            rawpos = P.tile([128, T8], f32, name="rawpos")
            rsums = P.tile([128, T8 * NCP], f32, name="rsums")
            sc8 = P.tile([128, T8], f32, name="sc8")
            pos8 = P.tile([128, T8], f32, name="pos8")
            ones = P.tile([128, 1], f32, name="ones")
            rbc = [P.tile([128, CGW], bf16, name=f"rbc_{c}") for c in range(CG)]
            erm = [None] * CG
            et = [[None] * CG for _ in range(KT)]
            etb = [None] * KT

            nc.vector.memset(ones[:], 1.0)

            def rm_load(dram_slice, name, tag, bufs):
                """[1024, 512] row-major DRAM rows -> [128, 8*512] tile
                (tile[p, t*512+d] = rows[t*128+p, d])."""
                sb = S.tile([128, T8 * DIM], bf16, name=name, tag=tag,
                            bufs=bufs)
                nc.sync.dma_start(
                    sb[:].rearrange("p (t d) -> p t d", d=DIM),
                    dram_slice.rearrange("(t p) d -> p t d", p=128))
                return sb

            def load_erm(c):
                erm[c] = rm_load(erm_d[c * CGW:(c + 1) * CGW, :],
                                 f"erm_{c}", "erm", 2)

            def load_et(c):
                for k in range(KT):
                    et[k][c] = P.tile([128, CGW], bf16, name=f"et_{k}_{c}")
                    nc.sync.dma_start(
                        et[k][c][:],
                        et_d[k * 128:(k + 1) * 128, c * CGW:(c + 1) * CGW])

            def sumsq(src, tt, dst, dcol, src2=None):
                sco = S.tile([128, DIM], bf16, tag="stt", name="sco")
                s2 = src2 if src2 is not None else src
                nc.vector.scalar_tensor_tensor(
                    sco[:], src[:, tt * DIM:(tt + 1) * DIM], 1.0,
                    s2[:, tt * DIM:(tt + 1) * DIM], ALU.mult, ALU.mult,
                    accum_out=dst[:, dcol:dcol + 1])

            def sumsq_cg(c):
                for tt in range(T8):
                    sumsq(erm[c], tt, ss64, c * 8 + tt)

            def rchain(c0, c1):
                """sqrt+recip+broadcast+normalize for column groups c0..c1-1
                (their ss64 columns must already be emitted)."""
                w = (c1 - c0) * T8
                nsq = S.tile([128, w], f32, tag=f"nsq{c0}", name="nsq")
                nc.scalar.activation(nsq[:], ss64[:, c0 * 8:c1 * 8], AF.Sqrt)
                rcp = S.tile([128, w], f32, tag=f"rcp{c0}", name="rcp")
                nc.vector.reciprocal(rcp[:], nsq[:])
                rcb = S.tile([128, w], bf16, tag=f"rcb{c0}", name="rcb")
                nc.vector.tensor_copy(rcb[:], rcp[:])
                for c in range(c0, c1):
                    off = (c - c0) * T8
                    nc.sync.dma_start(
                        bass.AP(rflat[c], 0, [[1, 128], [128, T8]]),
                        rcb[:, off:off + T8])
                    nc.sync.dma_start(rbc[c][:],
                                      bass.AP(rflat[c], 0, [[0, 128], [1, CGW]]))
                    for k in range(KT):
                        nc.vector.tensor_mul(et[k][c][:], et[k][c][:],
                                             rbc[c][:])

            def main_pair(cp):
                ca, cb = 2 * cp, 2 * cp + 1
                for t in range(T8):
                    ps = PS.tile([128, 2 * CGW], f32, tag="mm", name="psmm")
                    for ci, c in enumerate((ca, cb)):
                        for k in range(KT):
                            for n in range(NBF):
                                nc.tensor.matmul(
                                    ps[:, ci * CGW + n * 512:
                                       ci * CGW + (n + 1) * 512],
                                    etb[k][:, t * 128:(t + 1) * 128],
                                    et[k][c][:, n * 512:(n + 1) * 512],
                                    start=(k == 0), stop=(k == KT - 1))
                    sce = S.tile([128, 2 * CGW], bf16, tag="expout", name="sce")
                    col = t * NCP + cp
                    nc.scalar.activation(sce[:], ps[:], AF.Exp,
                                         scale=sc8[:, t:t + 1],
                                         accum_out=rsums[:, col:col + 1])

            def sumsq_act(src, tt, dst, dcol):
                sco = S.tile([128, DIM], bf16, tag="sqact", name="sco")
                nc.scalar.activation(sco[:], src[:, tt * DIM:(tt + 1) * DIM],
                                     AF.Square,
                                     accum_out=dst[:, dcol:dcol + 1])

            # ---- paced emission ----
            ermb = rm_load(ermb_d[:, :], "ermb", "ermb", 1)
            load_erm(0)
            load_et(0)
            for t in range(T8):
                sumsq(ermb, t, ssb, t)
            for k in range(KT):
                etb[k] = P.tile([128, RPC], bf16, name=f"etb_{k}")
                nc.sync.dma_start(etb[k][:], etb_d[k * 128:(k + 1) * 128, :])
            load_erm(1)
            load_et(1)
            for tt in range(T8):
                sumsq_act(erm[0], tt, ss64, tt)
            for tt in range(T8):
                sumsq_act(erm[1], tt, ss64, 8 + tt)

            nb8 = S.tile([128, T8], f32, tag="nrm", name="nb8")
            nc.scalar.activation(nb8[:], ssb[:], AF.Sqrt)
            rb8 = P.tile([128, T8], f32, name="rb8")
            nc.vector.reciprocal(rb8[:], nb8[:])
            nc.vector.tensor_scalar_mul(sc8[:], rb8[:], 1.0 / TEMP)

            rchain(0, 2)
            main_pair(0)

            load_erm(2)
            load_et(2)
            sumsq_cg(2)
            load_erm(3)
            load_et(3)
            sumsq_cg(3)
            rchain(2, 4)
            main_pair(1)

            ermp = rm_load(ermp_d[:, :], "ermp", "ermp", 1)
            load_erm(4)
            load_et(4)
            sumsq_cg(4)
            load_erm(5)
            load_et(5)
            sumsq_cg(5)
            for t in range(T8):
                sumsq(ermp, t, ssp, t)
            for t in range(T8):
                sumsq(ermb, t, rawpos, t, src2=ermp)
            load_erm(6)
            load_et(6)
            sumsq_cg(6)
            load_erm(7)
            load_et(7)
            sumsq_cg(7)

            np8 = S.tile([128, T8], f32, tag="nrm", name="np8")
            nc.scalar.activation(np8[:], ssp[:], AF.Sqrt)
            rp8 = P.tile([128, T8], f32, name="rp8")
            nc.vector.reciprocal(rp8[:], np8[:])
            pt0 = P.tile([128, T8], f32, name="pt0")
            nc.vector.tensor_mul(pt0[:], rawpos[:], rb8[:])
            pt1 = P.tile([128, T8], f32, name="pt1")
            nc.vector.tensor_mul(pt1[:], pt0[:], rp8[:])
            nc.vector.tensor_scalar_mul(pos8[:], pt1[:], 1.0 / TEMP)

            rchain(4, 8)
            main_pair(2)
            main_pair(3)

            # ---- finalize: den = rowsum - e^{1/T}; sum(log(den) - pos) ----
            den8 = P.tile([128, T8], f32, name="den8")
            nc.vector.tensor_reduce(
                den8[:], rsums[:].rearrange("p (t c) -> p t c", c=NCP),
                X, ALU.add)
            den8b = P.tile([128, T8], f32, name="den8b")
            nc.vector.tensor_scalar_add(den8b[:], den8[:], -EXP_DIAG)
            logd = S.tile([128, T8], f32, tag="logd", name="logd")
            tlog = P.tile([128, 1], f32, name="tlog")
            nc.scalar.activation(logd[:], den8b[:], AF.Ln, accum_out=tlog[:])
            tpos = P.tile([128, 1], f32, name="tpos")
            nc.vector.tensor_reduce(tpos[:], pos8[:], X, ALU.add)
            lv = P.tile([128, 1], f32, name="lv")
            nc.vector.tensor_sub(lv[:], tlog[:], tpos[:])
            psf = PS.tile([1, 1], f32, tag="mm", name="psf")
            nc.tensor.matmul(psf[:], lv[:], ones[:], start=True, stop=True)
            ob = P.tile([1, 1], f32, name="ob")
            nc.vector.tensor_copy(ob[:], psf[:])
            nc.sync.dma_start(out_d[:], ob[:])

    nc.compile()
    return nc


def _get_nc():
    if "nc" not in _CACHE:
        _CACHE["nc"] = _build()
    return _CACHE["nc"]


def _in_maps(emb_i, emb_j):
    bf = ml_dtypes.bfloat16
    E = np.concatenate([np.asarray(emb_i, dtype=np.float32),
                        np.asarray(emb_j, dtype=np.float32)], axis=0)
    Ebf = E.astype(bf)                              # [8192, 512] row-major
    ET = np.ascontiguousarray(Ebf.T)                # [512, 8192]
    maps = []
    for k in range(NCORES):
        s = k * RPC
        p = (s + BATCH) % B2
        maps.append({
            "et": ET,
            "etb": np.ascontiguousarray(ET[:, s:s + RPC]),
            "erm": Ebf,
            "ermb": np.ascontiguousarray(Ebf[s:s + RPC, :]),
            "ermp": np.ascontiguousarray(Ebf[p:p + RPC, :]),
        })
    return maps


def _run(emb_i, emb_j, trace=False):
    from concourse.bass_utils import run_bass_kernel_spmd
    nc = _get_nc()
    res = run_bass_kernel_spmd(nc, _in_maps(emb_i, emb_j),
                               list(range(NCORES)), trace=trace)
    total = sum(float(res.results[i]["out"][0, 0]) for i in range(NCORES))
    loss = np.float32(total / B2)
    return loss, res


def kernel(emb_i, emb_j):
    return _run(emb_i, emb_j, trace=False)[0]


# revision 8
# speedup vs baseline: 1.0281x; 1.0105x over previous
"""Trainium2 Bass kernel for NT-Xent contrastive loss (BATCH=4096, DIM=512, TEMP=0.5).

Strategy (data-parallel over rows of the 2B x 2B similarity matrix):
  - Host: E = concat(emb_i, emb_j) [8192, 512] f32, cast bf16. Each core gets
    ET = E.T (replicated) + its own / partner 1024-column blocks of ET, plus
    row-major copies (full + own + partner) for cheap DVE row-norms.
  - Device (per core, SPMD, no collectives):
      * sumsq of every row via DVE scalar_tensor_tensor square+accumulate
        over row-major tiles -> r = 1/||e|| (ACT sqrt in three batches paced
        with the DMA stream, DVE reciprocal)
      * normalize the rhs copy column-wise in place: z_j = e_j * r_j
        (r broadcast via DRAM-bounce + step-0 DMA)
      * S' = e_block^T @ Z on PE (bf16, fp32 accum), [128, 2048] PSUM groups
        (two column groups per exp)
      * ACT: exp(S' * r_row/TEMP) with fused row-sum accumulation
      * positives via DVE row-dots of own x partner row-major blocks
      * per-core partial: sum_rows(log(den - e^{1/TEMP}) - pos/TEMP) -> [1,1]
  - Host: loss = sum(partials) / (2B).

Emission order is deliberate: it sets per-engine queue order, pacing the
normalization pipeline (DMA -> DVE sumsq -> ACT sqrt -> DVE recip/cast ->
DMA broadcast -> DVE normalize) just ahead of the PE/ACT main-loop stream.
"""

import math

import ml_dtypes
import numpy as np

BATCH = 4096
DIM = 512
TEMP = 0.5
B2 = 2 * BATCH              # 8192 rows/cols of the similarity matrix
NCORES = 8
RPC = B2 // NCORES          # 1024 rows per core
KT = DIM // 128             # 4 contraction chunks
CG = 8                      # column groups
CGW = B2 // CG              # 1024 columns per group
T8 = RPC // 128             # 8 row-tiles per group / per core
NBF = CGW // 512            # 512-wide matmuls per group
NCP = CG // 2               # column-group pairs (exp granularity)
EXP_DIAG = math.exp(1.0 / TEMP)

_CACHE = {}


def _build():
    import concourse.bass as bass
    import concourse.bacc as bacc
    import concourse.mybir as mybir
    import concourse.tile as tile

    f32 = mybir.dt.float32
    bf16 = mybir.dt.bfloat16
    AF = mybir.ActivationFunctionType
    ALU = mybir.AluOpType
    X = mybir.AxisListType.X

    nc = bacc.Bacc("TRN2", target_bir_lowering=False, debug=False,
                   num_devices=NCORES)

    et_d = nc.dram_tensor("et", [DIM, B2], bf16, kind="ExternalInput").ap()
    etb_d = nc.dram_tensor("etb", [DIM, RPC], bf16, kind="ExternalInput").ap()
    erm_d = nc.dram_tensor("erm", [128, (B2 // 128) * DIM], bf16,
                           kind="ExternalInput").ap()
    ermb_d = nc.dram_tensor("ermb", [128, T8 * DIM], bf16,
                            kind="ExternalInput").ap()
    ermp_d = nc.dram_tensor("ermp", [128, T8 * DIM], bf16,
                            kind="ExternalInput").ap()
    out_d = nc.dram_tensor("out", [1, 1], f32, kind="ExternalOutput").ap()
    rflat = [nc.dram_tensor(f"rflat{c}", [CGW], bf16) for c in range(CG)]

    with tile.TileContext(nc) as tc:
        with (
            tc.tile_pool(name="persist", bufs=1) as P,
            tc.tile_pool(name="scratch", bufs=2) as S,
            tc.tile_pool(name="psum", bufs=2, space="PSUM") as PS,
        ):
            ss64 = P.tile([128, 64], f32, name="ss64")
            ssb = P.tile([128, T8], f32, name="ssb")
            ssp = P.tile([128, T8], f32, name="ssp")
            rawpos = P.tile([128, T8], f32, name="rawpos")
            rsums = P.tile([128, T8 * NCP], f32, name="rsums")
            sc8 = P.tile([128, T8], f32, name="sc8")
            pos8 = P.tile([128, T8], f32, name="pos8")
            ones = P.tile([128, 1], f32, name="ones")
            rbc = [P.tile([128, CGW], bf16, name=f"rbc_{c}") for c in range(CG)]
            erm = [None] * CG
            et = [[None] * CG for _ in range(KT)]
            etb = [None] * KT

            nc.vector.memset(ones[:], 1.0)

            def rm_load(dram_slice, name, tag, bufs):
                """Pre-tiled row-major chunk ([128, 8*512] SBUF image,
                tile[p, t*512+d] = row t*128+p, col d) -> one plain DMA."""
                sb = S.tile([128, T8 * DIM], bf16, name=name, tag=tag,
                            bufs=bufs)
                nc.sync.dma_start(sb[:], dram_slice)
                return sb

            def load_erm(c):
                erm[c] = rm_load(
                    erm_d[:, c * T8 * DIM:(c + 1) * T8 * DIM],
                    f"erm_{c}", "erm", 2)

            def load_et(c):
                for k in range(KT):
                    et[k][c] = P.tile([128, CGW], bf16, name=f"et_{k}_{c}")
                    nc.sync.dma_start(
                        et[k][c][:],
                        et_d[k * 128:(k + 1) * 128, c * CGW:(c + 1) * CGW])

            def sumsq(src, tt, dst, dcol, src2=None):
                sco = S.tile([128, DIM], bf16, tag="stt", name="sco")
                s2 = src2 if src2 is not None else src
                nc.vector.scalar_tensor_tensor(
                    sco[:], src[:, tt * DIM:(tt + 1) * DIM], 1.0,
                    s2[:, tt * DIM:(tt + 1) * DIM], ALU.mult, ALU.mult,
                    accum_out=dst[:, dcol:dcol + 1])

            def sumsq_cg(c):
                for tt in range(T8):
                    sumsq(erm[c], tt, ss64, c * 8 + tt)

            def rchain(c0, c1):
                """sqrt+recip+broadcast+normalize for column groups c0..c1-1
                (their ss64 columns must already be emitted)."""
                w = (c1 - c0) * T8
                nsq = S.tile([128, w], f32, tag=f"nsq{c0}", name="nsq")
                nc.scalar.activation(nsq[:], ss64[:, c0 * 8:c1 * 8], AF.Sqrt)
                rcp = S.tile([128, w], f32, tag=f"rcp{c0}", name="rcp")
                nc.vector.reciprocal(rcp[:], nsq[:])
                rcb = S.tile([128, w], bf16, tag=f"rcb{c0}", name="rcb")
                nc.vector.tensor_copy(rcb[:], rcp[:])
                for c in range(c0, c1):
                    off = (c - c0) * T8
                    nc.sync.dma_start(
                        bass.AP(rflat[c], 0, [[1, 128], [128, T8]]),
                        rcb[:, off:off + T8])
                    nc.sync.dma_start(rbc[c][:],
                                      bass.AP(rflat[c], 0, [[0, 128], [1, CGW]]))
                    for k in range(KT):
                        nc.vector.tensor_mul(et[k][c][:], et[k][c][:],
                                             rbc[c][:])

            def main_pair(cp):
                ca, cb = 2 * cp, 2 * cp + 1
                for t in range(T8):
                    ps = PS.tile([128, 2 * CGW], f32, tag="mm", name="psmm")
                    for ci, c in enumerate((ca, cb)):
                        for k in range(KT):
                            for n in range(NBF):
                                nc.tensor.matmul(
                                    ps[:, ci * CGW + n * 512:
                                       ci * CGW + (n + 1) * 512],
                                    etb[k][:, t * 128:(t + 1) * 128],
                                    et[k][c][:, n * 512:(n + 1) * 512],
                                    start=(k == 0), stop=(k == KT - 1))
                    sce = S.tile([128, 2 * CGW], bf16, tag="expout", name="sce")
                    col = t * NCP + cp
                    nc.scalar.activation(sce[:], ps[:], AF.Exp,
                                         scale=sc8[:, t:t + 1],
                                         accum_out=rsums[:, col:col + 1])

            def sumsq_act(src, tt, dst, dcol):
                sco = S.tile([128, DIM], bf16, tag="sqact", name="sco")
                nc.scalar.activation(sco[:], src[:, tt * DIM:(tt + 1) * DIM],
                                     AF.Square,
                                     accum_out=dst[:, dcol:dcol + 1])

            # ---- paced emission ----
            ermb = rm_load(ermb_d[:, :], "ermb", "ermb", 1)
            load_erm(0)
            load_et(0)
            for t in range(T8):
                sumsq(ermb, t, ssb, t)
            for k in range(KT):
                etb[k] = P.tile([128, RPC], bf16, name=f"etb_{k}")
                nc.sync.dma_start(etb[k][:], etb_d[k * 128:(k + 1) * 128, :])
            load_erm(1)
            load_et(1)
            for tt in range(T8):
                sumsq_act(erm[0], tt, ss64, tt)
            for tt in range(T8):
                sumsq_act(erm[1], tt, ss64, 8 + tt)

            nb8 = S.tile([128, T8], f32, tag="nrm", name="nb8")
            nc.scalar.activation(nb8[:], ssb[:], AF.Sqrt)
            rb8 = P.tile([128, T8], f32, name="rb8")
            nc.vector.reciprocal(rb8[:], nb8[:])
            nc.vector.tensor_scalar_mul(sc8[:], rb8[:], 1.0 / TEMP)

            rchain(0, 2)
            main_pair(0)

            load_erm(2)
            load_et(2)
            sumsq_cg(2)
            load_erm(3)
            load_et(3)
            sumsq_cg(3)
            rchain(2, 4)
            main_pair(1)

            ermp = rm_load(ermp_d[:, :], "ermp", "ermp", 1)
            load_erm(4)
            load_et(4)
            sumsq_cg(4)
            load_erm(5)
            load_et(5)
            sumsq_cg(5)
            for t in range(T8):
                sumsq(ermp, t, ssp, t)
            for t in range(T8):
                sumsq(ermb, t, rawpos, t, src2=ermp)
            load_erm(6)
            load_et(6)
            sumsq_cg(6)
            load_erm(7)
            load_et(7)
            sumsq_cg(7)

            np8 = S.tile([128, T8], f32, tag="nrm", name="np8")
            nc.scalar.activation(np8[:], ssp[:], AF.Sqrt)
            rp8 = P.tile([128, T8], f32, name="rp8")
            nc.vector.reciprocal(rp8[:], np8[:])
            pt0 = P.tile([128, T8], f32, name="pt0")
            nc.vector.tensor_mul(pt0[:], rawpos[:], rb8[:])
            pt1 = P.tile([128, T8], f32, name="pt1")
            nc.vector.tensor_mul(pt1[:], pt0[:], rp8[:])
            nc.vector.tensor_scalar_mul(pos8[:], pt1[:], 1.0 / TEMP)

            rchain(4, 8)
            main_pair(2)
            main_pair(3)

            # ---- finalize: den = rowsum - e^{1/T}; sum(log(den) - pos) ----
            den8 = P.tile([128, T8], f32, name="den8")
            nc.vector.tensor_reduce(
                den8[:], rsums[:].rearrange("p (t c) -> p t c", c=NCP),
                X, ALU.add)
            den8b = P.tile([128, T8], f32, name="den8b")
            nc.vector.tensor_scalar_add(den8b[:], den8[:], -EXP_DIAG)
            logd = S.tile([128, T8], f32, tag="logd", name="logd")
            tlog = P.tile([128, 1], f32, name="tlog")
            nc.scalar.activation(logd[:], den8b[:], AF.Ln, accum_out=tlog[:])
            tpos = P.tile([128, 1], f32, name="tpos")
            nc.vector.tensor_reduce(tpos[:], pos8[:], X, ALU.add)
            lv = P.tile([128, 1], f32, name="lv")
            nc.vector.tensor_sub(lv[:], tlog[:], tpos[:])
            psf = PS.tile([1, 1], f32, tag="mm", name="psf")
            nc.tensor.matmul(psf[:], lv[:], ones[:], start=True, stop=True)
            ob = P.tile([1, 1], f32, name="ob")
            nc.vector.tensor_copy(ob[:], psf[:])
            nc.sync.dma_start(out_d[:], ob[:])

    nc.compile()
    return nc


def _get_nc():
    if "nc" not in _CACHE:
        _CACHE["nc"] = _build()
    return _CACHE["nc"]


def _in_maps(emb_i, emb_j):
    bf = ml_dtypes.bfloat16
    E = np.concatenate([np.asarray(emb_i, dtype=np.float32),
                        np.asarray(emb_j, dtype=np.float32)], axis=0)
    Ebf = E.astype(bf)                              # [8192, 512] row-major
    ET = np.ascontiguousarray(Ebf.T)                # [512, 8192]
    # SBUF-image tiling of the row-major copy: ERMT[p, (t*512)+d] = Ebf[t*128+p, d]
    ERMT = np.ascontiguousarray(
        Ebf.reshape(B2 // 128, 128, DIM).transpose(1, 0, 2).reshape(128, -1))
    maps = []
    for k in range(NCORES):
        s = k * RPC
        p = (s + BATCH) % B2
        maps.append({
            "et": ET,
            "etb": np.ascontiguousarray(ET[:, s:s + RPC]),
            "erm": ERMT,
            "ermb": np.ascontiguousarray(
                ERMT[:, s // 128 * DIM:(s // 128 + T8) * DIM]),
            "ermp": np.ascontiguousarray(
                ERMT[:, p // 128 * DIM:(p // 128 + T8) * DIM]),
        })
    return maps


def _run(emb_i, emb_j, trace=False):
    from concourse.bass_utils import run_bass_kernel_spmd
    nc = _get_nc()
    res = run_bass_kernel_spmd(nc, _in_maps(emb_i, emb_j),
                               list(range(NCORES)), trace=trace)
    total = sum(float(res.results[i]["out"][0, 0]) for i in range(NCORES))
    loss = np.float32(total / B2)
    return loss, res


def kernel(emb_i, emb_j):
    return _run(emb_i, emb_j, trace=False)[0]


# revision 10
# speedup vs baseline: 1.2299x; 1.1963x over previous
"""Trainium2 Bass kernel for NT-Xent contrastive loss (BATCH=4096, DIM=512, TEMP=0.5).

Strategy (data-parallel over rows of the 2B x 2B similarity matrix):
  - Host: E = concat(emb_i, emb_j) [8192, 512] f32, cast bf16. Each core gets
    ET = E.T (replicated) + its own 1024-column block, plus row-major copies
    (full + own + partner) pre-tiled into the SBUF image layout, an identity
    and a row-selector constant.
  - Device (per core, SPMD, no collectives):
      * sumsq of every row via DVE scalar_tensor_tensor square+accumulate
      * r = 1/||e|| = exp(-0.5*ln(sumsq)) on ACT -- Exp and Ln share one
        activation table set, so the kernel never swaps tables
      * broadcast r across partitions with PE: transpose r-block via the
        tensor engine, then one selector matmul per row-tile
      * normalize the rhs copy column-wise in place: z_j = e_j * r_j (DVE)
      * S' = e_block^T @ Z on PE (bf16, fp32 accum); first two column groups
        as [128,1024] PSUM groups (early start), rest as [128,2048] pairs
      * ACT: exp(S' * r_row/TEMP) with fused row-sum accumulation
      * positives via DVE row-dots of own x partner row-major blocks
      * per-core partial: sum_rows(log(den - e^{1/TEMP}) - pos/TEMP) -> [1,1]
  - Host: loss = sum(partials) / (2B).

Emission order is deliberate: per-engine queue order paces the normalization
pipeline (DMA -> DVE sumsq -> ACT ln/exp -> PE broadcast -> DVE normalize)
just ahead of the PE/ACT main-loop stream.
"""

import math

import ml_dtypes
import numpy as np

BATCH = 4096
DIM = 512
TEMP = 0.5
B2 = 2 * BATCH              # 8192 rows/cols of the similarity matrix
NCORES = 8
RPC = B2 // NCORES          # 1024 rows per core
KT = DIM // 128             # 4 contraction chunks
CG = 8                      # column groups
CGW = B2 // CG              # 1024 columns per group
T8 = RPC // 128             # 8 row-tiles per group / per core
NBF = CGW // 512            # 512-wide matmuls per group
NG = 5                      # main groups per row-tile: c0, c1, cp1, cp2, cp3
EXP_DIAG = math.exp(1.0 / TEMP)

_CACHE = {}


def _build():
    import concourse.bacc as bacc
    import concourse.mybir as mybir
    import concourse.tile as tile

    f32 = mybir.dt.float32
    bf16 = mybir.dt.bfloat16
    AF = mybir.ActivationFunctionType
    ALU = mybir.AluOpType
    X = mybir.AxisListType.X

    nc = bacc.Bacc("TRN2", target_bir_lowering=False, debug=False,
                   num_devices=NCORES)

    et_d = nc.dram_tensor("et", [DIM, B2], bf16, kind="ExternalInput").ap()
    etb_d = nc.dram_tensor("etb", [DIM, RPC], bf16, kind="ExternalInput").ap()
    erm_d = nc.dram_tensor("erm", [128, (B2 // 128) * DIM], bf16,
                           kind="ExternalInput").ap()
    ermb_d = nc.dram_tensor("ermb", [128, T8 * DIM], bf16,
                            kind="ExternalInput").ap()
    ermp_d = nc.dram_tensor("ermp", [128, T8 * DIM], bf16,
                            kind="ExternalInput").ap()
    iden_d = nc.dram_tensor("iden", [128, 128], bf16, kind="ExternalInput").ap()
    sel_d = nc.dram_tensor("sel", [128, T8 * 128], bf16,
                           kind="ExternalInput").ap()
    out_d = nc.dram_tensor("out", [1, 1], f32, kind="ExternalOutput").ap()

    with tile.TileContext(nc) as tc:
        with (
            tc.tile_pool(name="persist", bufs=1) as P,
            tc.tile_pool(name="scratch", bufs=2) as S,
            tc.tile_pool(name="psum", bufs=2, space="PSUM") as PS,
        ):
            ss64 = P.tile([128, 64], f32, name="ss64")
            ssb = P.tile([128, T8], f32, name="ssb")
            ssp = P.tile([128, T8], f32, name="ssp")
            rawpos = P.tile([128, T8], f32, name="rawpos")
            rsums = P.tile([128, T8 * NG], f32, name="rsums")
            sc8 = P.tile([128, T8], f32, name="sc8")
            pos8 = P.tile([128, T8], f32, name="pos8")
            ones = P.tile([128, 1], f32, name="ones")
            iden = P.tile([128, 128], bf16, name="iden")
            sel = P.tile([128, T8 * 128], bf16, name="sel")
            rbc = [P.tile([128, CGW], bf16, name=f"rbc_{c}") for c in range(CG)]
            erm = [None] * CG
            et2 = [[None] * CG for _ in range(KT)]   # [k][c] -> [128, 1024]
            etb = [None] * KT

            nc.vector.memset(ones[:], 1.0)
            nc.sync.dma_start(iden[:], iden_d[:])
            nc.sync.dma_start(sel[:], sel_d[:])

            def load_rm(dram_ap, name):
                sb = P.tile([128, T8 * DIM], bf16, name=name)
                nc.sync.dma_start(sb[:], dram_ap)
                return sb

            def load_erm(c):
                erm[c] = load_rm(erm_d[:, c * T8 * DIM:(c + 1) * T8 * DIM],
                                 f"erm_{c}")

            def load_et(c):
                for k in range(KT):
                    et2[k][c] = P.tile([128, CGW], bf16, name=f"et_{k}_{c}")
                    nc.sync.dma_start(
                        et2[k][c][:],
                        et_d[k * 128:(k + 1) * 128, c * CGW:(c + 1) * CGW])

            def sumsq(src, tt, dst, dcol, src2=None):
                sco = S.tile([128, DIM], bf16, tag="stt", name="sco")
                s2 = src2 if src2 is not None else src
                nc.vector.scalar_tensor_tensor(
                    sco[:], src[:, tt * DIM:(tt + 1) * DIM], 1.0,
                    s2[:, tt * DIM:(tt + 1) * DIM], ALU.mult, ALU.mult,
                    accum_out=dst[:, dcol:dcol + 1])

            def rsqrt(dst, src_ap, w):
                """dst[:, 0:w] = 1/sqrt(src) via exp(-0.5*ln(x)) -- same ACT
                table set as the main-loop Exp, so no table swaps."""
                ln = S.tile([128, w], f32, tag=f"ln{w}", name="ln")
                nc.scalar.activation(ln[:], src_ap, AF.Ln)
                nc.scalar.activation(dst, ln[:], AF.Exp, scale=-0.5)

            def rchain(c):
                """r for group c -> broadcast via PE -> normalize in place."""
                rcb = S.tile([128, 128], bf16, tag="rcb", name="rcb")
                nc.vector.memset(rcb[:], 0.0)
                rsqrt(rcb[:, 0:T8], ss64[:, c * 8:(c + 1) * 8], T8)
                ptr = PS.tile([128, 128], bf16, tag="mm", name="ptr")
                nc.tensor.transpose(ptr[:], rcb[:], iden[:])
                rT = S.tile([128, 128], bf16, tag="rT", name="rT")
                nc.vector.tensor_copy(rT[:], ptr[:])
                pb = PS.tile([128, CGW], f32, tag="mm", name="pb")
                for t in range(T8):
                    nc.tensor.matmul(pb[:, t * 128:(t + 1) * 128],
                                     sel[:, t * 128:(t + 1) * 128],
                                     rT[:], start=True, stop=True)
                nc.vector.tensor_copy(rbc[c][:], pb[:])
                for k in range(KT):
                    nc.vector.tensor_mul(et2[k][c][:], et2[k][c][:],
                                         rbc[c][:])

            def main_group(gi, cgs):
                """One main group per row-tile over the given column groups."""
                for t in range(T8):
                    wid = len(cgs) * CGW
                    ps = PS.tile([128, wid], f32, tag="mm", name="psmm")
                    for k in range(KT):
                        for ci, c in enumerate(cgs):
                            for n in range(NBF):
                                lo = ci * CGW + n * 512
                                nc.tensor.matmul(
                                    ps[:, lo:lo + 512],
                                    etb[k][:, t * 128:(t + 1) * 128],
                                    et2[k][c][:, n * 512:(n + 1) * 512],
                                    start=(k == 0), stop=(k == KT - 1))
                    sce = S.tile([128, wid], bf16, tag="expout", name="sce")
                    col = t * NG + gi
                    nc.scalar.activation(sce[:], ps[:], AF.Exp,
                                         scale=sc8[:, t:t + 1],
                                         accum_out=rsums[:, col:col + 1])

            # ---- paced emission ----
            ermb = load_rm(ermb_d[:, :], "ermb")
            load_erm(0)
            load_erm(1)
            for t in range(T8):                      # own norms
                sumsq(ermb, t, ssb, t)
            for k in range(KT):
                etb[k] = P.tile([128, RPC], bf16, name=f"etb_{k}")
                nc.sync.dma_start(etb[k][:], etb_d[k * 128:(k + 1) * 128, :])
            load_et(0)
            load_et(1)
            for tt in range(T8):
                sumsq(erm[0], tt, ss64, tt)
            rb8 = P.tile([128, T8], f32, name="rb8")
            rsqrt(rb8[:], ssb[:], T8)
            nc.vector.tensor_scalar_mul(sc8[:], rb8[:], 1.0 / TEMP)
            rchain(0)
            main_group(0, (0,))

            load_erm(2)
            load_erm(3)
            load_et(2)
            load_et(3)
            for tt in range(T8):
                sumsq(erm[1], tt, ss64, 8 + tt)
            rchain(1)
            main_group(1, (1,))

            ermp = load_rm(ermp_d[:, :], "ermp")
            load_erm(4)
            load_erm(5)
            load_et(4)
            load_et(5)
            for c in (2, 3):
                for tt in range(T8):
                    sumsq(erm[c], tt, ss64, c * 8 + tt)
                rchain(c)
            main_group(2, (2, 3))

            load_erm(6)
            load_erm(7)
            load_et(6)
            load_et(7)
            for c in (4, 5):
                for tt in range(T8):
                    sumsq(erm[c], tt, ss64, c * 8 + tt)
                rchain(c)
            for c in (6, 7):
                for tt in range(T8):
                    sumsq(erm[c], tt, ss64, c * 8 + tt)
                rchain(c)
            main_group(3, (4, 5))

            for t in range(T8):                      # partner norms + positives
                sumsq(ermp, t, ssp, t)
            for t in range(T8):
                sumsq(ermb, t, rawpos, t, src2=ermp)
            rp8 = P.tile([128, T8], f32, name="rp8")
            rsqrt(rp8[:], ssp[:], T8)
            pt0 = P.tile([128, T8], f32, name="pt0")
            nc.vector.tensor_mul(pt0[:], rawpos[:], rb8[:])
            pt1 = P.tile([128, T8], f32, name="pt1")
            nc.vector.tensor_mul(pt1[:], pt0[:], rp8[:])
            nc.vector.tensor_scalar_mul(pos8[:], pt1[:], 1.0 / TEMP)

            main_group(4, (6, 7))

            # ---- finalize: den = rowsum - e^{1/T}; sum(log(den) - pos) ----
            den8 = P.tile([128, T8], f32, name="den8")
            nc.vector.tensor_reduce(
                den8[:], rsums[:].rearrange("p (t c) -> p t c", c=NG),
                X, ALU.add)
            den8b = P.tile([128, T8], f32, name="den8b")
            nc.vector.tensor_scalar_add(den8b[:], den8[:], -EXP_DIAG)
            logd = S.tile([128, T8], f32, tag="logd", name="logd")
            tlog = P.tile([128, 1], f32, name="tlog")
            nc.scalar.activation(logd[:], den8b[:], AF.Ln, accum_out=tlog[:])
            tpos = P.tile([128, 1], f32, name="tpos")
            nc.vector.tensor_reduce(tpos[:], pos8[:], X, ALU.add)
            lv = P.tile([128, 1], f32, name="lv")
            nc.vector.tensor_sub(lv[:], tlog[:], tpos[:])
            psf = PS.tile([1, 1], f32, tag="mm", name="psf")
            nc.tensor.matmul(psf[:], lv[:], ones[:], start=True, stop=True)
            ob = P.tile([1, 1], f32, name="ob")
            nc.vector.tensor_copy(ob[:], psf[:])
            nc.sync.dma_start(out_d[:], ob[:])

    nc.compile()
    return nc


def _get_nc():
    if "nc" not in _CACHE:
        _CACHE["nc"] = _build()
    return _CACHE["nc"]


def _in_maps(emb_i, emb_j):
    bf = ml_dtypes.bfloat16
    E = np.concatenate([np.asarray(emb_i, dtype=np.float32),
                        np.asarray(emb_j, dtype=np.float32)], axis=0)
    Ebf = E.astype(bf)                              # [8192, 512] row-major
    ET = np.ascontiguousarray(Ebf.T)                # [512, 8192]
    # SBUF-image tiling of the row-major copy: ERMT[p, t*512+d] = Ebf[t*128+p, d]
    ERMT = np.ascontiguousarray(
        Ebf.reshape(B2 // 128, 128, DIM).transpose(1, 0, 2).reshape(128, -1))
    SEL = np.zeros((128, T8 * 128), dtype=bf)
    for tp in range(T8):
        SEL[tp, tp * 128:(tp + 1) * 128] = 1.0
    maps = []
    for k in range(NCORES):
        s = k * RPC
        p = (s + BATCH) % B2
        maps.append({
            "et": ET,
            "etb": np.ascontiguousarray(ET[:, s:s + RPC]),
            "erm": ERMT,
            "ermb": np.ascontiguousarray(
                ERMT[:, s // 128 * DIM:(s // 128 + T8) * DIM]),
            "ermp": np.ascontiguousarray(
                ERMT[:, p // 128 * DIM:(p // 128 + T8) * DIM]),
            "iden": np.eye(128, dtype=bf),
            "sel": SEL,
        })
    return maps


def _run(emb_i, emb_j, trace=False):
    from concourse.bass_utils import run_bass_kernel_spmd
    nc = _get_nc()
    res = run_bass_kernel_spmd(nc, _in_maps(emb_i, emb_j),
                               list(range(NCORES)), trace=trace)
    total = sum(float(res.results[i]["out"][0, 0]) for i in range(NCORES))
    loss = np.float32(total / B2)
    return loss, res


def kernel(emb_i, emb_j):
    return _run(emb_i, emb_j, trace=False)[0]


# revision 11
# speedup vs baseline: 1.3359x; 1.0862x over previous
"""Trainium2 Bass kernel for NT-Xent contrastive loss (BATCH=4096, DIM=512, TEMP=0.5).

Strategy (data-parallel over rows of the 2B x 2B similarity matrix):
  - Host: E = concat(emb_i, emb_j) [8192, 512] f32, cast bf16. Each core gets
    ET = E.T (replicated) + its own 1024-column block, plus row-major copies
    (full + own + partner) pre-tiled into the SBUF image layout, an identity
    and a row-selector constant.
  - Device (per core, SPMD, no collectives):
      * sumsq of every row via DVE scalar_tensor_tensor square+accumulate
      * r = 1/||e|| = exp(-0.5*ln(sumsq)) on ACT -- Exp and Ln share one
        activation table set, so the kernel never swaps tables
      * broadcast r across partitions with PE: transpose r-block via the
        tensor engine, then one selector matmul per row-tile
      * normalize the rhs copy column-wise in place: z_j = e_j * r_j (DVE)
      * S' = e_block^T @ Z on PE (bf16, fp32 accum); first two column groups
        as [128,1024] PSUM groups (early start), rest as [128,2048] pairs
      * ACT: exp(S' * r_row/TEMP) with fused row-sum accumulation
      * positives via DVE row-dots of own x partner row-major blocks
      * per-core partial: sum_rows(log(den - e^{1/TEMP}) - pos/TEMP) -> [1,1]
  - Host: loss = sum(partials) / (2B).

Emission order is deliberate: per-engine queue order paces the normalization
pipeline (DMA -> DVE sumsq -> ACT ln/exp -> PE broadcast -> DVE normalize)
just ahead of the PE/ACT main-loop stream.
"""

import math

import ml_dtypes
import numpy as np

BATCH = 4096
DIM = 512
TEMP = 0.5
B2 = 2 * BATCH              # 8192 rows/cols of the similarity matrix
NCORES = 8
RPC = B2 // NCORES          # 1024 rows per core
KT = DIM // 128             # 4 contraction chunks
CG = 8                      # column groups
CGW = B2 // CG              # 1024 columns per group
T8 = RPC // 128             # 8 row-tiles per group / per core
NBF = CGW // 512            # 512-wide matmuls per group
NG = 5                      # main groups per row-tile: c0, c1, cp1, cp2, cp3
EXP_DIAG = math.exp(1.0 / TEMP)

_CACHE = {}


def _build():
    import concourse.bacc as bacc
    import concourse.mybir as mybir
    import concourse.tile as tile

    f32 = mybir.dt.float32
    bf16 = mybir.dt.bfloat16
    AF = mybir.ActivationFunctionType
    ALU = mybir.AluOpType
    X = mybir.AxisListType.X

    import bass_rust as _bass_rust
    from concourse.hw_specs import get_activation_tables

    class _Bacc(bacc.Bacc):
        """Bacc that pins Exp+Ln to the combined natural_log_exp_and_others
        activation-table set, so the kernel never swaps ACT tables."""

        def insert_act_table_loads(self):
            has_activation = any(
                isinstance(i, mybir.InstActivation)
                for b in self.main_func.blocks
                for i in b.instructions)
            if not has_activation:
                return
            drop = {mybir.ActivationFunctionType.Exp,
                    mybir.ActivationFunctionType.Ln}
            tables = []
            for name, funcs in get_activation_tables(self.m.arch).items():
                if name != "natural_log_exp_and_others":
                    funcs = funcs - drop
                tables.append((name, funcs))
            _bass_rust.insert_act_table_loads(self, tables)

    nc = _Bacc("TRN2", target_bir_lowering=False, debug=False,
               num_devices=NCORES)

    et_d = nc.dram_tensor("et", [DIM, B2], bf16, kind="ExternalInput").ap()
    etb_d = nc.dram_tensor("etb", [DIM, RPC], bf16, kind="ExternalInput").ap()
    erm_d = nc.dram_tensor("erm", [128, (B2 // 128) * DIM], bf16,
                           kind="ExternalInput").ap()
    ermb_d = nc.dram_tensor("ermb", [128, T8 * DIM], bf16,
                            kind="ExternalInput").ap()
    ermp_d = nc.dram_tensor("ermp", [128, T8 * DIM], bf16,
                            kind="ExternalInput").ap()
    iden_d = nc.dram_tensor("iden", [128, 128], bf16, kind="ExternalInput").ap()
    sel_d = nc.dram_tensor("sel", [128, T8 * 128], bf16,
                           kind="ExternalInput").ap()
    out_d = nc.dram_tensor("out", [1, 1], f32, kind="ExternalOutput").ap()

    with tile.TileContext(nc) as tc:
        with (
            tc.tile_pool(name="persist", bufs=1) as P,
            tc.tile_pool(name="scratch", bufs=2) as S,
            tc.tile_pool(name="psum", bufs=2, space="PSUM") as PS,
        ):
            ss64 = P.tile([128, 64], f32, name="ss64")
            ssb = P.tile([128, T8], f32, name="ssb")
            ssp = P.tile([128, T8], f32, name="ssp")
            rawpos = P.tile([128, T8], f32, name="rawpos")
            rsums = P.tile([128, T8 * NG], f32, name="rsums")
            sc8 = P.tile([128, T8], f32, name="sc8")
            pos8 = P.tile([128, T8], f32, name="pos8")
            ones = P.tile([128, 1], f32, name="ones")
            iden = P.tile([128, 128], bf16, name="iden")
            sel = P.tile([128, T8 * 128], bf16, name="sel")
            rbc = [P.tile([128, CGW], bf16, name=f"rbc_{c}") for c in range(CG)]
            erm = [None] * CG
            et2 = [[None] * CG for _ in range(KT)]   # [k][c] -> [128, 1024]
            etb = [None] * KT

            nc.vector.memset(ones[:], 1.0)
            nc.sync.dma_start(iden[:], iden_d[:])
            nc.sync.dma_start(sel[:], sel_d[:])

            def load_rm(dram_ap, name):
                sb = P.tile([128, T8 * DIM], bf16, name=name)
                nc.sync.dma_start(sb[:], dram_ap)
                return sb

            def load_erm(c):
                erm[c] = load_rm(erm_d[:, c * T8 * DIM:(c + 1) * T8 * DIM],
                                 f"erm_{c}")

            def load_et(c):
                for k in range(KT):
                    et2[k][c] = P.tile([128, CGW], bf16, name=f"et_{k}_{c}")
                    nc.sync.dma_start(
                        et2[k][c][:],
                        et_d[k * 128:(k + 1) * 128, c * CGW:(c + 1) * CGW])

            def sumsq(src, tt, dst, dcol, src2=None):
                sco = S.tile([128, DIM], bf16, tag="stt", name="sco")
                s2 = src2 if src2 is not None else src
                nc.vector.scalar_tensor_tensor(
                    sco[:], src[:, tt * DIM:(tt + 1) * DIM], 1.0,
                    s2[:, tt * DIM:(tt + 1) * DIM], ALU.mult, ALU.mult,
                    accum_out=dst[:, dcol:dcol + 1])

            def rsqrt(dst, src_ap, w):
                """dst[:, 0:w] = 1/sqrt(src) via exp(-0.5*ln(x)) -- same ACT
                table set as the main-loop Exp, so no table swaps."""
                ln = S.tile([128, w], f32, tag=f"ln{w}", name="ln")
                nc.scalar.activation(ln[:], src_ap, AF.Ln)
                nc.scalar.activation(dst, ln[:], AF.Exp, scale=-0.5)

            def rchain(c):
                """r for group c -> broadcast via PE -> normalize in place."""
                rcb = S.tile([128, 128], bf16, tag="rcb", name="rcb")
                nc.vector.memset(rcb[:], 0.0)
                rsqrt(rcb[:, 0:T8], ss64[:, c * 8:(c + 1) * 8], T8)
                ptr = PS.tile([128, 128], bf16, tag="mm", name="ptr")
                nc.tensor.transpose(ptr[:], rcb[:], iden[:])
                rT = S.tile([128, 128], bf16, tag="rT", name="rT")
                nc.vector.tensor_copy(rT[:], ptr[:])
                pb = PS.tile([128, CGW], f32, tag="mm", name="pb")
                for t in range(T8):
                    nc.tensor.matmul(pb[:, t * 128:(t + 1) * 128],
                                     sel[:, t * 128:(t + 1) * 128],
                                     rT[:], start=True, stop=True)
                nc.vector.tensor_copy(rbc[c][:], pb[:])
                for k in range(KT):
                    nc.vector.tensor_mul(et2[k][c][:], et2[k][c][:],
                                         rbc[c][:])

            def main_group(gi, cgs):
                """One main group per row-tile over the given column groups."""
                for t in range(T8):
                    wid = len(cgs) * CGW
                    ps = PS.tile([128, wid], f32, tag="mm", name="psmm")
                    for k in range(KT):
                        for ci, c in enumerate(cgs):
                            for n in range(NBF):
                                lo = ci * CGW + n * 512
                                nc.tensor.matmul(
                                    ps[:, lo:lo + 512],
                                    etb[k][:, t * 128:(t + 1) * 128],
                                    et2[k][c][:, n * 512:(n + 1) * 512],
                                    start=(k == 0), stop=(k == KT - 1))
                    sce = S.tile([128, wid], bf16, tag="expout", name="sce")
                    col = t * NG + gi
                    nc.scalar.activation(sce[:], ps[:], AF.Exp,
                                         scale=sc8[:, t:t + 1],
                                         accum_out=rsums[:, col:col + 1])

            # ---- paced emission ----
            ermb = load_rm(ermb_d[:, :], "ermb")
            load_erm(0)
            load_erm(1)
            for t in range(T8):                      # own norms
                sumsq(ermb, t, ssb, t)
            for k in range(KT):
                etb[k] = P.tile([128, RPC], bf16, name=f"etb_{k}")
                nc.sync.dma_start(etb[k][:], etb_d[k * 128:(k + 1) * 128, :])
            load_et(0)
            load_et(1)
            for tt in range(T8):
                sumsq(erm[0], tt, ss64, tt)
            rb8 = P.tile([128, T8], f32, name="rb8")
            rsqrt(rb8[:], ssb[:], T8)
            nc.vector.tensor_scalar_mul(sc8[:], rb8[:], 1.0 / TEMP)
            rchain(0)
            main_group(0, (0,))

            load_erm(2)
            load_erm(3)
            load_et(2)
            load_et(3)
            for tt in range(T8):
                sumsq(erm[1], tt, ss64, 8 + tt)
            rchain(1)
            main_group(1, (1,))

            ermp = load_rm(ermp_d[:, :], "ermp")
            load_erm(4)
            load_erm(5)
            load_et(4)
            load_et(5)
            for c in (2, 3):
                for tt in range(T8):
                    sumsq(erm[c], tt, ss64, c * 8 + tt)
                rchain(c)
            main_group(2, (2, 3))

            load_erm(6)
            load_erm(7)
            load_et(6)
            load_et(7)
            for c in (4, 5):
                for tt in range(T8):
                    sumsq(erm[c], tt, ss64, c * 8 + tt)
                rchain(c)
            for c in (6, 7):
                for tt in range(T8):
                    sumsq(erm[c], tt, ss64, c * 8 + tt)
                rchain(c)
            main_group(3, (4, 5))

            for t in range(T8):                      # partner norms + positives
                sumsq(ermp, t, ssp, t)
            for t in range(T8):
                sumsq(ermb, t, rawpos, t, src2=ermp)
            rp8 = P.tile([128, T8], f32, name="rp8")
            rsqrt(rp8[:], ssp[:], T8)
            pt0 = P.tile([128, T8], f32, name="pt0")
            nc.vector.tensor_mul(pt0[:], rawpos[:], rb8[:])
            pt1 = P.tile([128, T8], f32, name="pt1")
            nc.vector.tensor_mul(pt1[:], pt0[:], rp8[:])
            nc.vector.tensor_scalar_mul(pos8[:], pt1[:], 1.0 / TEMP)

            main_group(4, (6, 7))

            # ---- finalize: den = rowsum - e^{1/T}; sum(log(den) - pos) ----
            den8 = P.tile([128, T8], f32, name="den8")
            nc.vector.tensor_reduce(
                den8[:], rsums[:].rearrange("p (t c) -> p t c", c=NG),
                X, ALU.add)
            den8b = P.tile([128, T8], f32, name="den8b")
            nc.vector.tensor_scalar_add(den8b[:], den8[:], -EXP_DIAG)
            logd = S.tile([128, T8], f32, tag="logd", name="logd")
            tlog = P.tile([128, 1], f32, name="tlog")
            nc.scalar.activation(logd[:], den8b[:], AF.Ln, accum_out=tlog[:])
            tpos = P.tile([128, 1], f32, name="tpos")
            nc.vector.tensor_reduce(tpos[:], pos8[:], X, ALU.add)
            lv = P.tile([128, 1], f32, name="lv")
            nc.vector.tensor_sub(lv[:], tlog[:], tpos[:])
            psf = PS.tile([1, 1], f32, tag="mm", name="psf")
            nc.tensor.matmul(psf[:], lv[:], ones[:], start=True, stop=True)
            ob = P.tile([1, 1], f32, name="ob")
            nc.vector.tensor_copy(ob[:], psf[:])
            nc.sync.dma_start(out_d[:], ob[:])

    nc.compile()
    return nc


def _get_nc():
    if "nc" not in _CACHE:
        _CACHE["nc"] = _build()
    return _CACHE["nc"]


def _in_maps(emb_i, emb_j):
    bf = ml_dtypes.bfloat16
    E = np.concatenate([np.asarray(emb_i, dtype=np.float32),
                        np.asarray(emb_j, dtype=np.float32)], axis=0)
    Ebf = E.astype(bf)                              # [8192, 512] row-major
    ET = np.ascontiguousarray(Ebf.T)                # [512, 8192]
    # SBUF-image tiling of the row-major copy: ERMT[p, t*512+d] = Ebf[t*128+p, d]
    ERMT = np.ascontiguousarray(
        Ebf.reshape(B2 // 128, 128, DIM).transpose(1, 0, 2).reshape(128, -1))
    SEL = np.zeros((128, T8 * 128), dtype=bf)
    for tp in range(T8):
        SEL[tp, tp * 128:(tp + 1) * 128] = 1.0
    maps = []
    for k in range(NCORES):
        s = k * RPC
        p = (s + BATCH) % B2
        maps.append({
            "et": ET,
            "etb": np.ascontiguousarray(ET[:, s:s + RPC]),
            "erm": ERMT,
            "ermb": np.ascontiguousarray(
                ERMT[:, s // 128 * DIM:(s // 128 + T8) * DIM]),
            "ermp": np.ascontiguousarray(
                ERMT[:, p // 128 * DIM:(p // 128 + T8) * DIM]),
            "iden": np.eye(128, dtype=bf),
            "sel": SEL,
        })
    return maps


def _run(emb_i, emb_j, trace=False):
    from concourse.bass_utils import run_bass_kernel_spmd
    nc = _get_nc()
    res = run_bass_kernel_spmd(nc, _in_maps(emb_i, emb_j),
                               list(range(NCORES)), trace=trace)
    total = sum(float(res.results[i]["out"][0, 0]) for i in range(NCORES))
    loss = np.float32(total / B2)
    return loss, res


def kernel(emb_i, emb_j):
    return _run(emb_i, emb_j, trace=False)[0]
